# revision 57
# baseline (speedup 1.0000x reference)
"""Trainium2 Bass kernel for nn_Block_9268539425531 (MLA transformer block).

Sharding: 2 batch groups x 4-way TP within each group of 4 cores.
Per core (b = core//4, r = core%4, heads H = [4r, 4r+4)):
  Phase A: ln1 + w_down + kR(+rope) on own token slice (512 tokens);
           AllGather of dkv+kR issued early, then qR for own tokens and
           ALL 16 heads (token-major rope, fp8) overlaps it; a second
           small AllGather ships qR. No h gather needed at all.
  Phase B: q/k/v up-projections for own 4 heads, all 2048 tokens of own
           batch, SBUF-resident; q/k packed into fp8 DoubleRow score
           operands (ktile-0 = 128 dk, ktile-1 = rope dims).
  Phase C: causal attention for own 4 heads (scoresT layout; single fp8
           DoubleRow matmul per score block, contract 192; softmax
           denominator partials accumulated on the Pool engine with one
           final ones-matmul per (qi,h)); chunked 8-core AllGather of oT
           per query block (overlapped with attention compute).
  Phase D: w_o (fp8 DoubleRow over head-pair k-tiles) + residual + ln2 on
           own token slice; o gather is fp8.
  Phase E: FFN (full hidden dim, own token slice) + residual, computed as
           a 3-chain hi/lo fp8 DoubleRow decomposition
           (a_hi+a_lo)*w_hi + a_hi*(w_lo*16)/16 — near-bf16 accuracy at
           ~4x the bf16 matmul throughput per chain.
Exactly 2 AllGathers per iteration (collectives on this fabric are
latency-dominated: fewer, bigger, Shared-output gathers win).
fp32 PSUM accumulation everywhere.
"""
import math
import numpy as np
import ml_dtypes

B, T, C = 2, 2048, 2048
NH = 16
DK = 128
DHR = 64
LAT = 512
P = 128
NT = 512           # tokens per core
CC = C // P        # 16
NCORES = 8
SCALE = 1.0 / math.sqrt(DK)
NEG = -1.0e9
RG8 = [[0, 1, 2, 3, 4, 5, 6, 7]]
BF = ml_dtypes.bfloat16

_CACHE = {}


# ---------------------------------------------------------------- program ---
def build_program(repeat=1, nocc=False, stop_after=None, ag8=True,
                  shared=True, chunked=False, ownfirst=True, ffn8=False):
    from contextlib import ExitStack
    from concourse import bass, bacc, tile, mybir

    dt = mybir.dt
    f32 = dt.float32
    bf = dt.bfloat16
    f8 = dt.float8e4
    AF = mybir.ActivationFunctionType
    OP = mybir.AluOpType
    DR = mybir.MatmulPerfMode.DoubleRow

    nc = bacc.Bacc("TRN2", target_bir_lowering=False, debug=False,
                   num_devices=NCORES)

    def din(name, shape, dtype=bf):
        return nc.dram_tensor(name, shape, dtype, kind="ExternalInput")

    xT_d = din("xT", [CC, P, NT])
    ln1s_d = din("ln1s", [P, CC], f32)
    ln1b_d = din("ln1b", [P, CC], f32)
    ln2s_d = din("ln2s", [P, CC], f32)
    ln2b_d = din("ln2b", [P, CC], f32)
    wdown_d = din("wdown", [CC, P, 8 * P])
    bdown_d = din("bdown", [P, 8], f32)
    wqr_d = din("wqr", [CC, P, 8 * P])
    bqr_d = din("bqr", [P, 8], f32)
    wkr_d = din("wkr", [CC, P, P])
    bkr_d = din("bkr", [P, 1], f32)
    r2_d = din("r2", [P, P])
    cosq_d = din("cosq", [8, P, NT])
    sinq_d = din("sinq", [8, P, NT])
    cosk_d = din("cosk", [P, NT])
    sink_d = din("sink", [P, NT])
    wuk_d = din("wuk", [4, P, 4 * P])
    buk_d = din("buk", [P, 4], f32)
    wuv_d = din("wuv", [4, P, 4 * P])
    buv_d = din("buv", [P, 4], f32)
    wuq_d = din("wuq", [4, P, 4 * P])
    buq_d = din("buq", [P, 4], f32)
    mask_d = din("mask", [4, P, NT])
    ones_r_d = din("ones_r", [P, P])
    wo8_d = din("wo8", [4, 8, P, 2, 4 * P], f8)
    bo_d = din("bo", [P, CC], f32)
    wff1_d = din("wff1", [CC, 16, P, 4 * P])
    bff1_d = din("bff1", [P, 64], f32)
    wff2_d = din("wff2", [4, CC, P, CC * P])
    bff2_d = din("bff2", [P, CC], f32)
    f8e5 = dt.float8e5
    if ffn8:
        wff18_d = din("wff18", [8, 16, P, 2, 4 * P], f8)
        wff28_d = din("wff28", [4, 16, 8, P, 2, P], f8)
        wff1lo_d = din("wff1lo", [8, 16, P, 2, 4 * P], f8e5)
        wff2lo_d = din("wff2lo", [4, 16, 8, P, 2, P], f8e5)
    outT_d = nc.dram_tensor("outT", [CC, P, NT], f32, kind="ExternalOutput")

    with tile.TileContext(nc) as tc, ExitStack() as ctx:
        pc = ctx.enter_context(tc.tile_pool(name="const", bufs=1))
        pdram = ctx.enter_context(tc.tile_pool(name="dram", bufs=1, space="DRAM"))

        NS = NCORES if ag8 else 4
        RG = RG8 if ag8 else [[0, 1, 2, 3], [4, 5, 6, 7]]
        # dkv (8) + kR (1) + qR (8) in one gather: collectives are
        # latency-dominated on this fleet, so fewer+bigger wins
        agin1b_d = pdram.tile([17, P, NT], bf, name="agin1b")
        agin2_d = pdram.tile([4, 4, P, NT], f8, name="agin2")
        if chunked:
            # chunked per-qi AG2 out: Local (Shared allows only one writer)
            agout2_d = pdram.tile([4, NS, 4, P, NT], f8, name="agout2")

        # ---- small constants resident for the whole kernel
        ones_r = pc.tile([P, P], bf)
        nc.sync.dma_start(ones_r[:], ones_r_d[:])
        r2 = pc.tile([P, P], bf)
        nc.sync.dma_start(r2[:], r2_d[:])
        ln1s = pc.tile([P, CC], f32)
        nc.sync.dma_start(ln1s[:], ln1s_d[:])
        ln1b = pc.tile([P, CC], f32)
        nc.sync.dma_start(ln1b[:], ln1b_d[:])
        ln2s = pc.tile([P, CC], f32)
        nc.sync.dma_start(ln2s[:], ln2s_d[:])
        ln2b = pc.tile([P, CC], f32)
        nc.sync.dma_start(ln2b[:], ln2b_d[:])
        bdown = pc.tile([P, 8], f32)
        nc.sync.dma_start(bdown[:], bdown_d[:])
        bqr = pc.tile([P, 8], f32)
        nc.sync.dma_start(bqr[:], bqr_d[:])
        bkr = pc.tile([P, 1], f32)
        nc.sync.dma_start(bkr[:], bkr_d[:])
        buk = pc.tile([P, 4], f32)
        nc.sync.dma_start(buk[:], buk_d[:])
        buv = pc.tile([P, 4], f32)
        nc.sync.dma_start(buv[:], buv_d[:])
        buq = pc.tile([P, 4], f32)
        nc.sync.dma_start(buq[:], buq_d[:])
        bo = pc.tile([P, CC], f32)
        nc.sync.dma_start(bo[:], bo_d[:])
        bff1 = pc.tile([P, 64], f32)
        nc.sync.dma_start(bff1[:], bff1_d[:])
        bff2 = pc.tile([P, CC], f32)
        nc.sync.dma_start(bff2[:], bff2_d[:])
        masks = pc.tile([P, 4, NT], bf)
        nc.gpsimd.dma_start(masks[:], mask_d.transpose([1, 0, 2]))
        eps_t = pc.tile([P, 1], f32)
        nc.vector.memset(eps_t[:], 1e-6)

        pid = nc.partition_id()
        rr = pid % 4                      # rank in batch group
        # first slot of my batch group in gathered buffers
        soff = (pid // 4) * 4 if ag8 else 0

        def layer_norm(src_tiles, pstream, pstat, pool_ps, lns, lnb,
                       out_slices, out_name, rep):
            """src [16][P, NT] bf16 -> normalized bf16 slices."""
            ps_mean = pool_ps.tile([P, NT], f32, name=f"lnpm{rep}{out_name}")
            ps_sq = pool_ps.tile([P, NT], f32, name=f"lnps{rep}{out_name}")
            for ci in range(CC):
                sq = pstream.tile([P, NT], bf, name="lnsq", tag="lnsq")
                if ci % 2 == 0:
                    nc.vector.tensor_mul(sq[:], src_tiles[ci], src_tiles[ci])
                else:
                    nc.scalar.square(sq[:], src_tiles[ci])
                nc.tensor.matmul(ps_mean[:], ones_r[:], src_tiles[ci],
                                 start=(ci == 0), stop=(ci == CC - 1),
                                 skip_group_check=True)
                nc.tensor.matmul(ps_sq[:], ones_r[:], sq[:],
                                 start=(ci == 0), stop=(ci == CC - 1),
                                 skip_group_check=True)
            meanb = pstat.tile([P, NT], f32, name="lnmean", tag="lnmean")
            nc.vector.tensor_scalar_mul(meanb[:], ps_mean[:], 1.0 / C)
            m2 = pstat.tile([P, NT], f32, name="lnm2", tag="lnm2")
            nc.vector.tensor_mul(m2[:], meanb[:], meanb[:])
            var = pstat.tile([P, NT], f32, name="lnvar", tag="lnvar")
            nc.vector.scalar_tensor_tensor(var[:], ps_sq[:], 1.0 / C, m2[:],
                                           OP.mult, OP.subtract)
            std = pstat.tile([P, NT], f32, name="lnstd", tag="lnstd")
            nc.scalar.activation(std[:], var[:], AF.Sqrt, bias=eps_t[:])
            rstd = pstat.tile([P, NT], f32, name="lnrstd", tag="lnrstd")
            nc.vector.reciprocal(rstd[:], std[:])
            outs = []
            for ci in range(CC):
                eng = nc.gpsimd if ci % 4 == 3 else nc.vector
                t1 = pstream.tile([P, NT], f32, name="lnt1", tag="lnt1")
                eng.tensor_sub(t1[:], src_tiles[ci], meanb[:])
                t2 = pstream.tile([P, NT], f32, name="lnt2", tag="lnt2")
                eng.tensor_mul(t2[:], t1[:], rstd[:])
                h = out_slices[ci]
                eng.tensor_scalar(h, t2[:], lns[:, ci:ci + 1],
                                  lnb[:, ci:ci + 1], OP.mult, OP.add)
                outs.append(h)
            return outs

        _ph = ["A", "B", "C", "D", "E"]
        _upto = len(_ph) if stop_after is None else _ph.index(stop_after) + 1
        _en = set(_ph[:_upto])
        for rep in range(repeat):
            prep_cm = tc.tile_pool(name=f"prep{rep}", bufs=1)
            prep = prep_cm.__enter__()
            xTb = prep.tile([P, CC, NT], bf, name="xTb")
            # Shared DRAM tensors allow a single writing instruction each,
            # so allocate fresh AG outputs per rep.
            aspace = "Shared" if (shared and ag8) else "Local"
            agout1b_d = pdram.tile([NS, 17, P, NT], bf,
                                   name=f"agout1b{rep}", addr_space=aspace)
            if not chunked:
                agout2_d = pdram.tile([NS, 4, 4, P, NT], f8,
                                      name=f"agout2{rep}", addr_space=aspace)

            # ------------------------------------------------ phase A ----
            with (tc.tile_pool(name=f"pa{rep}", bufs=3) as pa,
                  tc.tile_pool(name=f"pas{rep}", bufs=1) as pas,
                  tc.tile_pool(name=f"pah{rep}", bufs=1) as pah,
                  tc.tile_pool(name=f"paw{rep}", bufs=20) as paw,
                  tc.tile_pool(name=f"paps{rep}", bufs=4, space="PSUM") as paps,
                  tc.tile_pool(name=f"papr{rep}", bufs=2, space="PSUM") as papr,
                  tc.tile_pool(name=f"past{rep}", bufs=1, space="PSUM") as pstat):
                for ci in range(CC):
                    (nc.sync if ci % 2 == 0 else nc.scalar).dma_start(
                        xTb[:, ci, :], xT_d[ci])
                # weight prefetch first (scalar queue; gpsimd is reserved
                # for collectives so an AG wait never blocks a prefetch)
                wd_tiles = []
                for ci in range(CC):
                    w = paw.tile([P, 8 * P], bf, name="wdt", tag="wdt")
                    (nc.scalar if ci % 2 == 0 else nc.sync).dma_start(
                        w[:], wdown_d[ci])
                    wd_tiles.append(w)
                wkr_sb = []
                for ci in range(CC):
                    w = paw.tile([P, P], bf, name="wkrt", tag="wkrt")
                    nc.scalar.dma_start(w[:], wkr_d[ci])
                    wkr_sb.append(w)
                cosk = pa.tile([P, NT], bf, name="coskt")
                nc.scalar.dma_start(cosk[:], cosk_d[:])
                sink = pa.tile([P, NT], bf, name="sinkt")
                nc.scalar.dma_start(sink[:], sink_d[:])
                wqr_sb = []
                for ci in range(CC):
                    # reuses the wdt ring: w_down tiles are fully consumed
                    # by the dkv matmuls before qR needs these
                    w = paw.tile([P, 8 * P], bf, name="wqrt", tag="wdt")
                    (nc.scalar if ci % 2 == 0 else nc.sync).dma_start(
                        w[:], wqr_d[ci])
                    wqr_sb.append(w)
                xT = [xTb[:, ci, :] for ci in range(CC)]
                hb = pah.tile([P, CC, NT], bf, name="hb")
                hts = layer_norm(xT, pa, pas, pstat, ln1s, ln1b,
                                 [hb[:, ci, :] for ci in range(CC)], "h", rep)
                # w_down projection (8 out chunks) on own tokens; ci-outer in
                # groups of 4 so the PE consumes LN output chunks as the DVE
                # emits them instead of waiting for all 16.
                dkvb = pah.tile([P, 8, NT], bf, name="dkvb")
                for mg in range(2):
                    pss4 = [paps.tile([P, NT], f32, name=f"psdkv{mg}{i}",
                                      tag="psdkv") for i in range(4)]
                    for ci in range(CC):
                        for i in range(4):
                            mi = 4 * mg + i
                            nc.tensor.matmul(
                                pss4[i][:], wd_tiles[ci][:, mi * P:(mi + 1) * P],
                                hts[ci],
                                start=(ci == 0), stop=(ci == CC - 1))
                    for i in range(4):
                        nc.vector.tensor_scalar_add(dkvb[:, 4 * mg + i, :],
                                                    pss4[i][:],
                                                    bdown[:, 4 * mg + i:4 * mg + i + 1])
                # kR on own tokens (+rope), shipped with dkv
                pskr = paps.tile([P, NT], f32, name="pskr", tag="psdkv")
                for ci in range(CC):
                    nc.tensor.matmul(pskr[:], wkr_sb[ci][:], hts[ci],
                                     start=(ci == 0), stop=(ci == CC - 1))
                krpre = pa.tile([P, NT], bf, name="krpre")
                nc.scalar.activation(krpre[:], pskr[:], AF.Identity,
                                     bias=bkr[:, 0:1])
                rotk = papr.tile([P, NT], f32, name="psrotk", tag="psrot")
                nc.tensor.matmul(rotk[:], r2[:], krpre[:], start=True,
                                 stop=True)
                tmpk = pa.tile([P, NT], f32, name="tmpk")
                nc.vector.tensor_mul(tmpk[:], rotk[:], sink[:])
                tmpk2 = pa.tile([P, NT], f32, name="tmpk2")
                nc.vector.tensor_mul(tmpk2[:], krpre[:], cosk[:])
                krb = pa.tile([P, NT], bf, name="krb")
                nc.vector.tensor_add(krb[:], tmpk2[:], tmpk[:])
                nc.sync.dma_start(agin1b_d[0:8].transpose([1, 0, 2]), dkvb[:])
                nc.sync.dma_start(agin1b_d[8], krb[:])
                # qR for own tokens, ALL 16 heads (token-major rope), so phase
                # B never needs gathered h; ships in the same gather.
                qrb = pah.tile([P, 8, NT], bf, name="qrb")
                for mg in range(2):
                    pres4 = []
                    for i in range(4):
                        mq = 4 * mg + i
                        ps = paps.tile([P, NT], f32, name=f"psqra{mq}",
                                       tag="psdkv")
                        for ci in range(CC):
                            nc.tensor.matmul(
                                ps[:], wqr_sb[ci][:, mq * P:(mq + 1) * P],
                                hts[ci], start=(ci == 0), stop=(ci == CC - 1))
                        pre = pa.tile([P, NT], bf, name="qrpre", tag="qrpre")
                        nc.scalar.activation(pre[:], ps[:], AF.Identity,
                                             bias=bqr[:, mq:mq + 1])
                        pres4.append((mq, pre))
                    for mq, pre in pres4:
                        rot = papr.tile([P, NT], f32, name=f"psrotq{mq}",
                                        tag="psrot")
                        nc.tensor.matmul(rot[:], r2[:], pre[:], start=True,
                                         stop=True)
                        cos_t = pa.tile([P, NT], bf, name="cosqa", tag="cosqa")
                        nc.scalar.dma_start(cos_t[:], cosq_d[mq])
                        sin_t = pa.tile([P, NT], bf, name="sinqa", tag="sinqa")
                        nc.scalar.dma_start(sin_t[:], sinq_d[mq])
                        tmp = pa.tile([P, NT], f32, name="rtmpa", tag="rtmpa")
                        nc.vector.tensor_mul(tmp[:], rot[:], sin_t[:])
                        tmp2 = pa.tile([P, NT], f32, name="rtmpb", tag="rtmpb")
                        nc.vector.tensor_mul(tmp2[:], pre[:], cos_t[:])
                        nc.vector.tensor_add(qrb[:, mq, :], tmp2[:], tmp[:])
                nc.sync.dma_start(agin1b_d[9:17].transpose([1, 0, 2]), qrb[:])
            if nocc:
                nc.sync.dma_start(agout1b_d[0], agin1b_d[:])
            else:
                nc.gpsimd.collective_compute(
                    "AllGather", mybir.AluOpType.bypass, replica_groups=RG,
                    ins=[agin1b_d[:].opt()], outs=[agout1b_d[:].opt()])

            # ------------------------------------------------ phase B ----
            if "B" not in _en:
                prep_cm.__exit__(None, None, None)
                continue
            pdw_cm = tc.tile_pool(name=f"pdw{rep}", bufs=1)
            pdw = pdw_cm.__enter__()
            pprod_cm = tc.tile_pool(name=f"prod{rep}", bufs=1)
            pprod = pprod_cm.__enter__()
            # fp8 DoubleRow packs: ktile-0 = q/k (128 dims), ktile-1 = rope
            # dims on rows 0-63 (k side rows 64-127 zeroed; q side garbage
            # there multiplies the zeros)
            qp = [pprod.tile([P, 2, 4, NT], f8, name=f"qp_{m}")
                  for m in range(4)]
            kp = [pprod.tile([P, 2, 4, NT], f8, name=f"kp_{m}")
                  for m in range(4)]
            vt = [pprod.tile([P, 4 * P], bf, name=f"vt_{i}") for i in range(16)]
            for m in range(4):
                nc.vector.memset(kp[m][64:128, 1, :, :], 0.0)
                # q-side pad rows multiply the k-side zeros, but uninit fp8
                # bytes can be NaN and NaN*0=NaN in the PE — zero them too
                nc.vector.memset(qp[m][64:128, 1, :, :], 0.0)
            with (tc.tile_pool(name=f"pbw{rep}", bufs=1) as pw,
                  tc.tile_pool(name=f"pbh{rep}", bufs=2) as pbh,
                  tc.tile_pool(name=f"pbps{rep}", bufs=3, space="PSUM") as pps):
                wuq_sb = []
                wuk_sb = []
                wuv_sb = []
                for lc in range(4):
                    w = pw.tile([P, 4 * P], bf, name=f"wuq{lc}")
                    nc.sync.dma_start(w[:], wuq_d[lc])
                    wuq_sb.append(w)
                    w = pw.tile([P, 4 * P], bf, name=f"wuk{lc}")
                    nc.scalar.dma_start(w[:], wuk_d[lc])
                    wuk_sb.append(w)
                    w = pw.tile([P, 4 * P], bf, name=f"wuv{lc}")
                    nc.sync.dma_start(w[:], wuv_d[lc])
                    wuv_sb.append(w)

                for nt in range(4):
                    slot = soff + nt
                    ckvb = pbh.tile([P, 11, NT], bf, name="ckvb", tag="ckvb")
                    nc.scalar.dma_start(
                        ckvb[:, 0:9, :],
                        agout1b_d[bass.ds(slot, 1)][0][0:9]
                        .transpose([1, 0, 2]))
                    # own-head qR planes from the same gather
                    nc.sync.dma_start(
                        ckvb[:, 9:11, :],
                        agout1b_d[bass.ds(slot, 1),
                                  bass.ds(9 + 2 * rr, 2)][0]
                        .transpose([1, 0, 2]))
                    for m in range(4):
                        nc.vector.tensor_copy(kp[m][0:64, 1, nt, :],
                                              ckvb[0:64, 8, :])
                        nc.vector.tensor_copy(
                            qp[m][0:64, 1, nt, :],
                            ckvb[64 * (m % 2):64 * (m % 2) + 64,
                                 9 + m // 2, :])
                    # q / k up-projections (4 chunks each)
                    for mt in range(4):
                        ps = pps.tile([P, NT], f32, name="psq", tag="psqr")
                        for lc in range(4):
                            nc.tensor.matmul(
                                ps[:], wuq_sb[lc][:, mt * P:(mt + 1) * P],
                                ckvb[:, 4 + lc, :],
                                start=(lc == 0), stop=(lc == 3))
                        nc.vector.tensor_scalar_add(qp[mt][:, 0, nt, :], ps[:],
                                                    buq[:, mt:mt + 1])
                        ps2 = pps.tile([P, NT], f32, name="psk", tag="psqr")
                        for lc in range(4):
                            nc.tensor.matmul(
                                ps2[:], wuk_sb[lc][:, mt * P:(mt + 1) * P],
                                ckvb[:, lc, :],
                                start=(lc == 0), stop=(lc == 3))
                        nc.vector.tensor_scalar_add(kp[mt][:, 0, nt, :], ps2[:],
                                                    buk[:, mt:mt + 1])
                    # v (token-major), bias deferred to phase C
                    for tt in range(4):
                        ps = pps.tile([P, 4 * P], f32, name="psv", tag="psqr")
                        for lc in range(4):
                            nc.tensor.matmul(
                                ps[:], ckvb[:, lc, tt * P:(tt + 1) * P],
                                wuv_sb[lc][:], start=(lc == 0), stop=(lc == 3))
                        nc.vector.tensor_copy(vt[4 * nt + tt][:], ps[:])

            # ---------------------------------------------- phase C ----
            if "C" not in _en:
                pprod_cm.__exit__(None, None, None)
                pdw_cm.__exit__(None, None, None)
                prep_cm.__exit__(None, None, None)
                continue
            wog = []
            for mig in range(1):
                wgm = []
                for t2 in range(8):
                    wg = pdw.tile([P, 2, 4 * P], f8, name=f"wog{mig}{t2}")
                    (nc.sync if t2 % 2 == 0 else nc.scalar).dma_start(
                        wg[:], wo8_d[mig, t2])
                    wgm.append(wg)
                wog.append(wgm)
            with (tc.tile_pool(name=f"pce{rep}", bufs=8) as pex,
                  tc.tile_pool(name=f"pco{rep}", bufs=3) as pot,
                  tc.tile_pool(name=f"pca{rep}", bufs=3) as pacc,
                  tc.tile_pool(name=f"pcps{rep}", bufs=4, space="PSUM") as pcsc,
                  tc.tile_pool(name=f"pcpo{rep}", bufs=2, space="PSUM") as pcso,
                  tc.tile_pool(name=f"pcpm{rep}", bufs=2, space="PSUM") as pcss):
                # software-pipelined (depth 3): scores for step s+2 issue on
                # PE before the PV matmuls of step s, so the PE never waits
                # on the mask-add (DVE) + exp (ACT) chain.
                steps = [(qi, h, ki) for qi in range(4) for h in range(4)
                         for ki in range(4 * qi + 4)]
                DEPTH = 3
                grp = {}

                def issue_scores(qi, h, ki):
                    kb, kc = divmod(ki, 4)
                    kcs = slice(kc * P, (kc + 1) * P)
                    psc = pcsc.tile([P, NT], f32, name="psc", tag="psc")
                    nc.tensor.matmul(psc[:], kp[h][:, :, kb, kcs],
                                     qp[h][:, :, qi, :], start=True, stop=True,
                                     perf_mode=DR)
                    if ki >= 4 * qi:
                        nc.vector.tensor_add(psc[:], psc[:],
                                             masks[:, ki - 4 * qi, :])
                    ex = pex.tile([P, NT], bf, name="ex", tag="ex")
                    nc.scalar.activation(ex[:], psc[:], AF.Exp, scale=SCALE)
                    return ex

                def issue_pv(qi, h, ki, ex):
                    nki = 4 * qi + 4
                    if ki == 0:
                        pso = pcso.tile([P, NT], f32, name="pso", tag="pso")
                        exa = pacc.tile([P, NT], f32, name="exa", tag="exa")
                        exa2 = pacc.tile([P, NT], f32, name="exa2", tag="exa2")
                        grp[(qi, h)] = (pso, exa, exa2)
                    pso, exa, exa2 = grp[(qi, h)]
                    nc.tensor.matmul(pso[:], vt[ki][:, h * P:(h + 1) * P],
                                     ex[:], start=(ki == 0),
                                     stop=(ki == nki - 1))
                    # denominator partials: two interleaved DVE accumulators
                    # (halves chain latency); one matmul per (qi,h) contracts
                    acc = exa if ki % 2 == 0 else exa2
                    if ki < 2:
                        nc.vector.tensor_copy(acc[:], ex[:])
                    else:
                        nc.vector.tensor_add(acc[:], acc[:], ex[:])
                    if ki == nki - 1:
                        exb = pot.tile([P, NT], bf, name="exb", tag="exb")
                        nc.vector.tensor_add(exb[:], exa[:], exa2[:])
                        pss = pcss.tile([P, NT], f32, name="pss", tag="pss")
                        nc.tensor.matmul(pss[:], ones_r[:], exb[:],
                                         start=True, stop=True)
                        rec = pot.tile([P, NT], f32, name="rec", tag="rec")
                        nc.vector.reciprocal(rec[:], pss[:])
                        ot = pot.tile([P, NT], f32, name="ot", tag="ot")
                        nc.vector.tensor_mul(ot[:], pso[:], rec[:])
                        otb = pot.tile([P, NT], f8, name="otb", tag="otb")
                        nc.vector.tensor_scalar_add(otb[:], ot[:],
                                                    buv[:, h:h + 1])
                        nc.sync.dma_start(agin2_d[qi, h], otb[:])
                        if h == 3 and chunked:
                            if nocc:
                                nc.sync.dma_start(agout2_d[qi, 0],
                                                  agin2_d[qi])
                            else:
                                nc.gpsimd.collective_compute(
                                    "AllGather", mybir.AluOpType.bypass,
                                    replica_groups=RG,
                                    ins=[agin2_d[qi].opt()],
                                    outs=[agout2_d[qi].opt()])

                pend = []
                for s, (qi, h, ki) in enumerate(steps):
                    ex = issue_scores(qi, h, ki)
                    pend.append((qi, h, ki, ex))
                    if len(pend) >= DEPTH:
                        issue_pv(*pend.pop(0))
                for args in pend:
                    issue_pv(*args)
                if not chunked:
                    if nocc:
                        nc.sync.dma_start(agout2_d[0], agin2_d[:])
                    else:
                        nc.gpsimd.collective_compute(
                            "AllGather", mybir.AluOpType.bypass,
                            replica_groups=RG,
                            ins=[agin2_d[:].opt()],
                            outs=[agout2_d[:].opt()])
            pprod_cm.__exit__(None, None, None)

            # ------------------------------------------------ phase D ----
            if "D" not in _en:
                pdw_cm.__exit__(None, None, None)
                prep_cm.__exit__(None, None, None)
                continue
            with tc.tile_pool(name=f"pde{rep}", bufs=1) as pper:
                with (tc.tile_pool(name=f"pdo{rep}", bufs=1) as pdo,
                      tc.tile_pool(name=f"pdd{rep}", bufs=12) as pdd,
                      tc.tile_pool(name=f"pdt{rep}", bufs=3) as pdt,
                      tc.tile_pool(name=f"pds{rep}", bufs=1) as pds,
                      tc.tile_pool(name=f"pdps{rep}", bufs=4, space="PSUM") as pdps,
                      tc.tile_pool(name=f"pdst{rep}", bufs=1, space="PSUM") as pdst):
                    otb_ = pdo.tile([P, 4, 4, NT], f8, name="otb_")
                    for j in range(4):
                        if chunked:
                            osrc = agout2_d[bass.ds(rr, 1),
                                            bass.ds(soff + j, 1)][0, 0]
                        else:
                            osrc = agout2_d[bass.ds(soff + j, 1),
                                            bass.ds(rr, 1)][0, 0]
                        nc.sync.dma_start(otb_[:, j, :, :],
                                          osrc.transpose([1, 0, 2]))
                    for mig in range(1, 4):
                        wgm = []
                        for t2 in range(8):
                            wg = pdd.tile([P, 2, 4 * P], f8, name="wogl",
                                          tag="wogl")
                            (nc.sync if t2 % 2 == 0 else nc.scalar).dma_start(
                                wg[:], wo8_d[mig, t2])
                            wgm.append(wg)
                        wog.append(wgm)
                    xmid = []
                    for mig in range(4):
                        ps4 = [pdps.tile([P, NT], f32, name=f"pswo{mig}{i}",
                                         tag="pswo") for i in range(4)]
                        for t2 in range(8):
                            osl = otb_[:, t2 // 2,
                                       (2 * t2) % 4:(2 * t2) % 4 + 2, :]
                            for ml in range(4):
                                nc.tensor.matmul(
                                    ps4[ml][:],
                                    wog[mig][t2][:, :, ml * P:(ml + 1) * P],
                                    osl, start=(t2 == 0), stop=(t2 == 7),
                                    perf_mode=DR)
                        for ml in range(4):
                            mi = mig * 4 + ml
                            xm = pper.tile([P, NT], bf, name=f"xmid{mi}")
                            nc.vector.scalar_tensor_tensor(
                                xm[:], ps4[ml][:], bo[:, mi:mi + 1],
                                xTb[:, mi, :], OP.add, OP.add)
                            xmid.append(xm)
                    h2_tiles = [pper.tile([P, NT], bf, name=f"h2_{ci}")
                                for ci in range(CC)]
                    h2 = layer_norm([t[:] for t in xmid], pdt, pds, pdst,
                                    ln2s, ln2b,
                                    [t[:] for t in h2_tiles], "h2_", rep)

                # -------------------------------------------- phase E ----
                if "E" not in _en:
                    pass
                elif ffn8:
                 # fp8 DoubleRow FFN, hi/lo 3-chain for near-bf16 accuracy:
                 # a*w ~= a_hi*w_hi + a_lo*w_hi + a_hi*w_lo, all three
                 # accumulating into ONE psum (w_lo is the unscaled e5m2
                 # residual, so no post-scale pass is needed); the dropped
                 # a_lo*w_lo term is ~0.2% of the result.
                 with (tc.tile_pool(name=f"pew{rep}", bufs=9) as pew,
                      tc.tile_pool(name=f"pewl{rep}", bufs=9) as pewl,
                      tc.tile_pool(name=f"pew2{rep}", bufs=3) as pew2,
                      tc.tile_pool(name=f"peg{rep}", bufs=2) as peg,
                      tc.tile_pool(name=f"pegt{rep}", bufs=4) as pegt,
                      tc.tile_pool(name=f"pea{rep}", bufs=1) as pea,
                      tc.tile_pool(name=f"pet{rep}", bufs=3) as pet,
                      tc.tile_pool(name=f"peps{rep}", bufs=4, space="PSUM") as peps,
                      tc.tile_pool(name=f"pep2{rep}", bufs=2, space="PSUM") as pep2):
                    h28 = pea.tile([P, CC, NT], f8, name="h28")
                    h28lo = pea.tile([P, CC, NT], f8, name="h28lo")
                    for ci in range(CC):
                        nc.scalar.copy(h28[:, ci, :], h2[ci])
                        nc.vector.tensor_sub(h28lo[:, ci, :], h2[ci],
                                             h28[:, ci, :])
                    accs = [pea.tile([P, NT], f32, name=f"ffacc{mi}")
                            for mi in range(16)]
                    for hb2 in range(4):
                        gt8 = peg.tile([P, 16, NT], f8, name="gt8", tag="gt8")
                        gt8lo = peg.tile([P, 16, NT], f8, name="gt8lo",
                                         tag="gt8lo")
                        for mtg in range(4):
                            mtg_g = hb2 * 4 + mtg
                            wts = []
                            wtsl = []
                            for kk in range(8):
                                w = pew.tile([P, 2, 4 * P], f8, name="wf18",
                                             tag="wf18")
                                nc.sync.dma_start(w[:], wff18_d[kk, mtg_g])
                                wts.append(w)
                                wl = pewl.tile([P, 2, 4 * P], f8e5,
                                               name="wf1l", tag="wf1l")
                                nc.scalar.dma_start(wl[:], wff1lo_d[kk, mtg_g])
                                wtsl.append(wl)
                            for ml in range(4):
                                psm = peps.tile([P, NT], f32,
                                                name=f"psf1{mtg}{ml}",
                                                tag="psf1")
                                for kk in range(8):
                                    nc.tensor.matmul(
                                        psm[:],
                                        wts[kk][:, :, ml * P:(ml + 1) * P],
                                        h28[:, 2 * kk:2 * kk + 2, :],
                                        start=(kk == 0), stop=False,
                                        perf_mode=DR)
                                for kk in range(8):
                                    nc.tensor.matmul(
                                        psm[:],
                                        wts[kk][:, :, ml * P:(ml + 1) * P],
                                        h28lo[:, 2 * kk:2 * kk + 2, :],
                                        start=False, stop=False,
                                        perf_mode=DR)
                                for kk in range(8):
                                    nc.tensor.matmul(
                                        psm[:],
                                        wtsl[kk][:, :, ml * P:(ml + 1) * P],
                                        h28[:, 2 * kk:2 * kk + 2, :],
                                        start=False, stop=(kk == 7),
                                        perf_mode=DR)
                                mt = mtg_g * 4 + ml
                                gtb = pegt.tile([P, NT], bf, name="gtb",
                                                tag="gtb")
                                nc.scalar.activation(gtb[:], psm[:],
                                                     AF.Gelu_apprx_tanh,
                                                     bias=bff1[:, mt:mt + 1])
                                sl = 4 * mtg + ml
                                nc.vector.tensor_copy(gt8[:, sl, :], gtb[:])
                                nc.vector.tensor_sub(gt8lo[:, sl, :], gtb[:],
                                                     gt8[:, sl, :])
                        for mi in range(16):
                            w2 = pew2.tile([P, 8, 2, P], f8, name="wf28",
                                           tag="wf28")
                            nc.sync.dma_start(w2[:], wff28_d[hb2, mi])
                            w2l = pew2.tile([P, 8, 2, P], f8e5, name="wf2l",
                                            tag="wf2l")
                            nc.scalar.dma_start(w2l[:], wff2lo_d[hb2, mi])
                            ps2 = pep2.tile([P, NT], f32, name="psf2",
                                            tag="psf2")
                            for kk in range(8):
                                nc.tensor.matmul(
                                    ps2[:], w2[:, kk, :, :],
                                    gt8[:, 2 * kk:2 * kk + 2, :],
                                    start=(kk == 0), stop=False,
                                    perf_mode=DR)
                            for kk in range(8):
                                nc.tensor.matmul(
                                    ps2[:], w2[:, kk, :, :],
                                    gt8lo[:, 2 * kk:2 * kk + 2, :],
                                    start=False, stop=False,
                                    perf_mode=DR)
                            for kk in range(8):
                                nc.tensor.matmul(
                                    ps2[:], w2l[:, kk, :, :],
                                    gt8[:, 2 * kk:2 * kk + 2, :],
                                    start=False, stop=(kk == 7),
                                    perf_mode=DR)
                            if hb2 == 0:
                                nc.vector.tensor_copy(accs[mi][:], ps2[:])
                            else:
                                nc.vector.tensor_add(accs[mi][:], accs[mi][:],
                                                     ps2[:])
                    for mi in range(CC):
                        ob = pet.tile([P, NT], f32, name="outb", tag="outb")
                        nc.vector.scalar_tensor_tensor(
                            ob[:], accs[mi][:], bff2[:, mi:mi + 1],
                            xmid[mi][:], OP.add, OP.add)
                        nc.sync.dma_start(outT_d[mi], ob[:])
                else:
                 with (tc.tile_pool(name=f"pew{rep}", bufs=17) as pew,
                      tc.tile_pool(name=f"pew2{rep}", bufs=3) as pew2,
                      tc.tile_pool(name=f"peg{rep}", bufs=17) as peg,
                      tc.tile_pool(name=f"pea{rep}", bufs=1) as pea,
                      tc.tile_pool(name=f"pet{rep}", bufs=3) as pet,
                      tc.tile_pool(name=f"peps{rep}", bufs=4, space="PSUM") as peps,
                      tc.tile_pool(name=f"pep2{rep}", bufs=2, space="PSUM") as pep2):
                    accs = [pea.tile([P, NT], f32, name=f"ffacc{mi}")
                            for mi in range(16)]
                    for hb2 in range(4):
                        gts = []
                        for mtg in range(4):
                            mtg_g = hb2 * 4 + mtg
                            wts = []
                            for ci in range(CC):
                                w = pew.tile([P, 4 * P], bf, name="wf1",
                                             tag="wf1")
                                nc.sync.dma_start(w[:], wff1_d[ci, mtg_g])
                                wts.append(w)
                            ps4 = [peps.tile([P, NT], f32,
                                             name=f"psf1{mtg}{i}", tag="psf1")
                                   for i in range(4)]
                            for ci in range(CC):
                                for ml in range(4):
                                    nc.tensor.matmul(
                                        ps4[ml][:],
                                        wts[ci][:, ml * P:(ml + 1) * P],
                                        h2[ci],
                                        start=(ci == 0), stop=(ci == CC - 1))
                            for ml in range(4):
                                mt = mtg_g * 4 + ml
                                gt = peg.tile([P, NT], bf, name="gt", tag="gt")
                                nc.scalar.activation(gt[:], ps4[ml][:],
                                                     AF.Gelu_apprx_tanh,
                                                     bias=bff1[:, mt:mt + 1])
                                gts.append(gt)
                        for mi in range(16):
                            w2 = pew2.tile([P, CC * P], bf, name="wf2",
                                           tag="wf2")
                            nc.sync.dma_start(w2[:], wff2_d[hb2, mi])
                            ps2 = pep2.tile([P, NT], f32, name="psf2",
                                            tag="psf2")
                            for hl in range(16):
                                nc.tensor.matmul(
                                    ps2[:], w2[:, hl * P:(hl + 1) * P],
                                    gts[hl][:],
                                    start=(hl == 0), stop=(hl == 15))
                            if hb2 == 0:
                                nc.vector.tensor_copy(accs[mi][:], ps2[:])
                            else:
                                nc.vector.tensor_add(accs[mi][:], accs[mi][:],
                                                     ps2[:])
                    for mi in range(CC):
                        ob = pet.tile([P, NT], f32, name="outb", tag="outb")
                        nc.vector.scalar_tensor_tensor(
                            ob[:], accs[mi][:], bff2[:, mi:mi + 1],
                            xmid[mi][:], OP.add, OP.add)
                        nc.sync.dma_start(outT_d[mi], ob[:])
            pdw_cm.__exit__(None, None, None)
            prep_cm.__exit__(None, None, None)

    nc.compile()
    return nc


# ------------------------------------------------------------------ host ---
def _rope_tables(r):
    """cos/sin tiles for core rank r; q tables token-major (own 512 tokens,
    all 16 heads: chunk mq holds heads 2mq, 2mq+1)."""
    t = np.arange(NT * r, NT * (r + 1), dtype=np.float64) + 1.0   # own tokens
    l = np.arange(DHR)
    cosq = np.zeros((8, P, NT), np.float64)
    sinq = np.zeros((8, P, NT), np.float64)
    for mq in range(8):
        for hl in range(2):
            h = 2 * mq + hl
            theta = 10000.0 ** (-2.0 * (32 * h + l // 2) / 1024.0)
            ang = t[None, :] * theta[:, None]            # [64, NT]
            cosq[mq, 64 * hl:64 * hl + 64] = np.cos(ang)
            sinq[mq, 64 * hl:64 * hl + 64] = np.sin(ang)
    thk = 10000.0 ** (-2.0 * (l // 2) / 64.0)
    angk = t[None, :] * thk[:, None]
    cosk = np.concatenate([np.cos(angk)] * 2, axis=0)     # [128, NT]
    sink = np.concatenate([np.sin(angk)] * 2, axis=0)
    return (np.ascontiguousarray(cosq, BF), np.ascontiguousarray(sinq, BF),
            np.ascontiguousarray(cosk, BF), np.ascontiguousarray(sink, BF))


def _shared_consts():
    r2 = np.zeros((P, P), np.float32)
    for i in range(64):
        r2[2 * i + 1, 2 * i] = -1.0
        r2[2 * i, 2 * i + 1] = 1.0
    mask = np.zeros((4, P, NT), np.float32)
    kl = np.arange(P)[:, None]
    ql = np.arange(NT)[None, :]
    for j in range(4):
        mask[j] = np.where(P * j + kl > ql, NEG, 0.0)
    ones = np.ones((P, P), np.float32)
    return r2.astype(BF), mask.astype(BF), ones.astype(BF)


def prepare_in_maps(inputs):
    f = np.float32
    g = {k: np.asarray(v, f) for k, v in inputs.items()}
    x = g["x"]
    r2, mask, ones = _shared_consts()

    def bfc(a):
        return np.ascontiguousarray(a).astype(BF)

    wdown_t = bfc(g["w_down"].reshape(CC, P, 8 * P))
    bdown_t = np.ascontiguousarray(g["b_down"].reshape(8, P).T)
    wkr2 = np.concatenate([g["w_kr"], g["w_kr"]], axis=1)  # [C, 128]
    wkr_t = bfc(wkr2.reshape(CC, P, P))
    bkr_t = np.ascontiguousarray(
        np.concatenate([g["b_kr"], g["b_kr"]]).reshape(P, 1))
    bo_t = np.ascontiguousarray(g["b_o"].reshape(CC, P).T)
    wff1_t = bfc(g["w_ff1"].reshape(CC, P, 16, 4 * P).transpose(0, 2, 1, 3))
    bff1_t = np.ascontiguousarray(g["b_ff1"].reshape(64, P).T)
    wff2_t = bfc(g["w_ff2"].reshape(4, CC, P, CC, P).transpose(0, 3, 2, 1, 4)
                 .reshape(4, CC, P, CC * P))
    F8 = ml_dtypes.float8_e4m3
    wo8_t = np.ascontiguousarray(
        g["w_o"].reshape(8, 2, P, 4, 4 * P).transpose(3, 0, 2, 1, 4)
    ).astype(F8)
    F8E5 = ml_dtypes.float8_e5m2
    w1r = np.ascontiguousarray(
        g["w_ff1"].reshape(8, 2, P, 16, 4 * P).transpose(0, 3, 2, 1, 4))
    wff18_t = w1r.astype(F8)
    wff1lo_t = (w1r - wff18_t.astype(np.float32)).astype(F8E5)
    w2r = np.ascontiguousarray(
        g["w_ff2"].reshape(4, 8, 2, P, CC, P).transpose(0, 4, 1, 3, 2, 5))
    wff28_t = w2r.astype(F8)
    wff2lo_t = (w2r - wff28_t.astype(np.float32)).astype(F8E5)
    bff2_t = np.ascontiguousarray(g["b_ff2"].reshape(CC, P).T)
    ln1s_t = np.ascontiguousarray(g["ln1_scale"].reshape(CC, P).T)
    ln1b_t = np.ascontiguousarray(g["ln1_bias"].reshape(CC, P).T)
    ln2s_t = np.ascontiguousarray(g["ln2_scale"].reshape(CC, P).T)
    ln2b_t = np.ascontiguousarray(g["ln2_bias"].reshape(CC, P).T)

    in_maps = []
    for c in range(NCORES):
        b, r = divmod(c, 4)
        cosq, sinq, cosk, sink = _rope_tables(r)
        xs = x[b, NT * r:NT * (r + 1), :].T                      # [C, NT]
        xT_t = bfc(xs.reshape(CC, P, NT))
        wuk_c = g["w_ukv"][:, 512 * r:512 * (r + 1)]
        wuv_c = g["w_ukv"][:, C + 512 * r:C + 512 * (r + 1)]
        wuq_c = g["w_uq"][:, 512 * r:512 * (r + 1)]
        m = {
            "xT": xT_t,
            "ln1s": ln1s_t, "ln1b": ln1b_t, "ln2s": ln2s_t, "ln2b": ln2b_t,
            "wdown": wdown_t, "bdown": bdown_t,
            "wqr": bfc(g["w_qr"].reshape(CC, P, 8 * P)),
            "bqr": np.ascontiguousarray(g["b_qr"].reshape(8, P).T),
            "wkr": wkr_t, "bkr": bkr_t,
            "r2": r2,
            "cosq": cosq, "sinq": sinq, "cosk": cosk, "sink": sink,
            "wuk": bfc(wuk_c.reshape(4, P, 4 * P)),
            "buk": np.ascontiguousarray(
                g["b_ukv"][512 * r:512 * (r + 1)].reshape(4, P).T),
            "wuv": bfc(wuv_c.reshape(4, P, 4 * P)),
            "buv": np.ascontiguousarray(
                g["b_ukv"][C + 512 * r:C + 512 * (r + 1)].reshape(4, P).T),
            "wuq": bfc(wuq_c.reshape(4, P, 4 * P)),
            "buq": np.ascontiguousarray(
                g["b_uq"][512 * r:512 * (r + 1)].reshape(4, P).T),
            "mask": mask, "ones_r": ones,
            "wo8": wo8_t, "bo": bo_t,
            "wff1": wff1_t, "bff1": bff1_t,
            "wff2": wff2_t, "bff2": bff2_t,
            "wff18": wff18_t, "wff28": wff28_t,
            "wff1lo": wff1lo_t, "wff2lo": wff2lo_t,
        }
        in_maps.append(m)
    return in_maps


def assemble_output(results):
    out = np.zeros((B, T, C), np.float32)
    for c in range(NCORES):
        b, r = divmod(c, 4)
        o = results[c]["outT"].reshape(C, NT)
        out[b, NT * r:NT * (r + 1), :] = o.T
    return out


def kernel(**inputs):
    from concourse import bass_utils
    nc = _CACHE.get("nc")
    if nc is None:
        nc = build_program(repeat=1)
        _CACHE["nc"] = nc
    in_maps = prepare_in_maps(inputs)
    res = bass_utils.run_bass_kernel_spmd(nc, in_maps,
                                          core_ids=list(range(NCORES)))
    return assemble_output(res.results)

